# revision 1
# baseline (speedup 1.0000x reference)
"""Causal attention (single head, d=1024) on 8 trn2 NeuronCores.

Problem: x[4,2048,1024], Wq/Wk/Wv[1024,1024] fp32;
out = softmax(mask(QK^T)/sqrt(1024)) @ V with mask j <= i+1.

Sharding: 2 cores per batch. Causal row work grows ~linearly with row
index, so the two cores split the 16 row-blocks of 128 as
{g : g%4 in {0,3}} vs {g : g%4 in {1,2}} (balanced). Each core receives
x[b] with its own rows permuted to the front so that every core runs the
same SPMD program; causality is enforced by a per-core additive mask
tensor (data, not code). K/V are computed redundantly per core (no
collectives).

Precision: logits have std ~1024 and softmax temperature 1, so scores
need ~2^-16 relative accuracy or argmax flips corrupt rows. The Q/K/S
chain therefore uses 3-term split-bf16 matmuls (hi/lo decomposition,
error ~2^-17); V is computed with f32r matmuls and stored bf16; P
(attention weights, ~one-hot) is bf16.

Structure: phase 0 loads x row-blocks, PE-transposes them, computes V
immediately from a transient f32r copy, and spills x^T as bf16 hi/lo
pairs to per-chunk DRAM scratch tensors (fine-grained dependencies so
later passes overlap). Q and K projection passes stream x^T back per
512-column chunk; attention row-blocks run last.
"""

import numpy as np
import ml_dtypes

import concourse.bass as bass
import concourse.mybir as mybir
import concourse.tile as tile
from concourse import bacc, masks
from concourse.bass_utils import run_bass_kernel_spmd

B, S, D, DA = 4, 2048, 1024, 1024
NCORES = 8
NBLK = S // 128  # 16 row blocks per batch
F32 = mybir.dt.float32
F32R = mybir.dt.float32r
BF16 = mybir.dt.bfloat16

ABLK = [g for g in range(NBLK) if g % 4 in (0, 3)]
BBLK = [g for g in range(NBLK) if g % 4 in (1, 2)]

NEG = -1e30


def _perm_rows(my):
    oth = [g for g in range(NBLK) if g not in my]
    idx = []
    for g in my + oth:
        idx.extend(range(g * 128, (g + 1) * 128))
    return np.array(idx, dtype=np.int64)


def _chunk_schedule():
    """Per local row-block l: which 512-col chunks of the permuted S row
    must be computed (union over the two roles, so the program is SPMD)."""
    sched = []
    for l in range(8):
        need = [False] * 4
        for my in (ABLK, BBLK):
            perm = _perm_rows(my)  # permuted col -> global row
            jmax = my[l] * 128 + 127 + 1  # max attended global col
            attended = perm <= jmax
            for ch in range(4):
                if attended[ch * 512 : (ch + 1) * 512].any():
                    need[ch] = True
        sched.append([ch for ch in range(4) if need[ch]])
    return sched


CHUNKS = _chunk_schedule()

_CACHE = {}


def _build():
    if "nc" in _CACHE:
        return _CACHE["nc"]

    nc = bacc.Bacc()
    x_d = nc.dram_tensor("x_perm", [S, D], F32, kind="ExternalInput")
    wq_d = nc.dram_tensor("wq", [D, DA], F32, kind="ExternalInput")
    wk_d = nc.dram_tensor("wk", [D, DA], F32, kind="ExternalInput")
    wv_d = nc.dram_tensor("wv", [D, DA], F32, kind="ExternalInput")
    mask_d = nc.dram_tensor("maskb", [1024, S], BF16, kind="ExternalInput")
    out_d = nc.dram_tensor("out", [1024, DA], F32, kind="ExternalOutput")
    # x^T spill: one tensor per 512-col chunk (fine-grained deps)
    xth_d = [nc.dram_tensor(f"xth{jc}", [D, 512], BF16) for jc in range(4)]
    xtl_d = [nc.dram_tensor(f"xtl{jc}", [D, 512], BF16) for jc in range(4)]

    from contextlib import ExitStack

    with tile.TileContext(nc) as tc, ExitStack() as stack:
        cpool = stack.enter_context(tc.tile_pool(name="const", bufs=1))
        ident = cpool.tile([128, 128], F32, tag="ident")
        masks.make_identity(nc, ident[:])

        # long-lived residents (live until the end of attention)
        vpool = stack.enter_context(tc.tile_pool(name="vres", bufs=1))
        V = [vpool.tile([128, DA], BF16, name=f"v{j}", tag=f"v{j}") for j in range(16)]
        qpool = stack.enter_context(tc.tile_pool(name="qtres", bufs=1))
        QTh = [qpool.tile([128, 1024], BF16, name=f"qth{a}", tag=f"qth{a}") for a in range(8)]
        QTl = [qpool.tile([128, 1024], BF16, name=f"qtl{a}", tag=f"qtl{a}") for a in range(8)]
        kpool = stack.enter_context(tc.tile_pool(name="ktres", bufs=1))
        KTh = [kpool.tile([128, S], BF16, name=f"kth{a}", tag=f"kth{a}") for a in range(8)]
        KTl = [kpool.tile([128, S], BF16, name=f"ktl{a}", tag=f"ktl{a}") for a in range(8)]

        # ---- Phase 0: transpose x, compute V, spill x^T hi/lo -------------
        with (
            tc.tile_pool(name="ph0w", bufs=1) as p0w,
            tc.tile_pool(name="ph0x", bufs=1) as p0x,
            tc.tile_pool(name="ph0", bufs=2) as p0,
            tc.tile_pool(name="ph0ps", bufs=2, space="PSUM") as p0ps,
            tc.tile_pool(name="ph0psv", bufs=4, space="PSUM") as p0psv,
        ):
            wv = [p0w.tile([128, DA], F32R, name=f"wv{d}", tag=f"wv{d}") for d in range(8)]
            for d in range(8):
                nc.gpsimd.dma_start(wv[d][:], wv_d[d * 128 : (d + 1) * 128, :])

            for jc in range(4):  # groups of 4 row-blocks (512 rows)
                xn = [p0x.tile([128, D], F32, name=f"xn{i}", tag=f"xn{i}") for i in range(4)]
                for i in range(4):
                    r0 = (jc * 4 + i) * 128
                    nc.sync.dma_start(xn[i][:], x_d[r0 : r0 + 128, :])
                xtr = [p0x.tile([128, 512], F32R, name=f"xtr{d}", tag=f"xtr{d}") for d in range(8)]
                for dc in range(8):
                    pst = p0ps.tile([128, 512], F32, tag="pst")
                    for i in range(4):
                        nc.tensor.transpose(
                            pst[:, i * 128 : (i + 1) * 128],
                            xn[i][:, dc * 128 : (dc + 1) * 128],
                            ident[:],
                        )
                    hsb = p0.tile([128, 512], BF16, tag="hsb")
                    lsb = p0.tile([128, 512], BF16, tag="lsb")
                    nc.vector.tensor_copy(hsb[:], pst[:])
                    nc.vector.tensor_sub(lsb[:], pst[:], hsb[:])
                    nc.vector.tensor_copy(xtr[dc][:], pst[:])
                    dsl = slice(dc * 128, (dc + 1) * 128)
                    nc.sync.dma_start(xth_d[jc][dsl, :], hsb[:])
                    nc.sync.dma_start(xtl_d[jc][dsl, :], lsb[:])
                # V for this group of 4 row-blocks
                for q in range(4):
                    vj = jc * 4 + q
                    for half in range(2):
                        ps = p0psv.tile([128, 512], F32, tag="ps")
                        for d in range(8):
                            nc.tensor.matmul(
                                ps[:],
                                xtr[d][:, q * 128 : (q + 1) * 128],
                                wv[d][:, half * 512 : (half + 1) * 512],
                                start=(d == 0),
                                stop=(d == 7),
                            )
                        nc.vector.tensor_copy(
                            V[vj][:, half * 512 : (half + 1) * 512], ps[:]
                        )

        # ---- Phase 1: Q^T then K^T (hi/lo bf16, 3-pass) -------------------
        def load_w_hilo(whpool, stpool, w_d):
            wh = [whpool.tile([128, DA], BF16, name=f"wh{d}", tag=f"wh{d}") for d in range(8)]
            wl = [whpool.tile([128, DA], BF16, name=f"wl{d}", tag=f"wl{d}") for d in range(8)]
            for d in range(8):
                nc.gpsimd.dma_start(wh[d][:], w_d[d * 128 : (d + 1) * 128, :])
                wst = stpool.tile([128, DA], F32, tag="wst")
                nc.sync.dma_start(wst[:], w_d[d * 128 : (d + 1) * 128, :])
                nc.vector.tensor_sub(wl[d][:], wst[:], wh[d][:])
            return wh, wl

        def load_xt_hilo(pool, jc):
            xh = [pool.tile([128, 512], BF16, name=f"xh{d}", tag=f"xh{d}") for d in range(8)]
            xl = [pool.tile([128, 512], BF16, name=f"xl{d}", tag=f"xl{d}") for d in range(8)]
            for d in range(8):
                dsl = slice(d * 128, (d + 1) * 128)
                nc.scalar.dma_start(xh[d][:], xth_d[jc][dsl, :])
                nc.scalar.dma_start(xl[d][:], xtl_d[jc][dsl, :])
            return xh, xl

        def pass_3term(wh, wl, xh, xl, ps):
            for d in range(8):
                for ac in range(8):
                    whs = wh[d][:, ac * 128 : (ac + 1) * 128]
                    wls = wl[d][:, ac * 128 : (ac + 1) * 128]
                    nc.tensor.matmul(ps[ac][:], whs, xh[d][:], start=(d == 0), stop=False)
                    nc.tensor.matmul(ps[ac][:], whs, xl[d][:], start=False, stop=False)
                    nc.tensor.matmul(ps[ac][:], wls, xh[d][:], start=False, stop=(d == 7))

        with (
            tc.tile_pool(name="phqw", bufs=1) as pqw,
            tc.tile_pool(name="phqst", bufs=2) as pqst,
            tc.tile_pool(name="phqx", bufs=2) as pqx,
            tc.tile_pool(name="phqps", bufs=1, space="PSUM") as pqps,
        ):
            wh, wl = load_w_hilo(pqw, pqst, wq_d)
            for jc in range(2):
                csl = slice(jc * 512, (jc + 1) * 512)
                xh, xl = load_xt_hilo(pqx, jc)
                ps = [pqps.tile([128, 512], F32, name=f"ps{a}", tag=f"ps{a}") for a in range(8)]
                pass_3term(wh, wl, xh, xl, ps)
                for ac in range(8):
                    nc.vector.tensor_copy(QTh[ac][:, csl], ps[ac][:])
                    nc.vector.tensor_sub(QTl[ac][:, csl], ps[ac][:], QTh[ac][:, csl])

        with (
            tc.tile_pool(name="phkw", bufs=1) as pkw,
            tc.tile_pool(name="phkst", bufs=2) as pkst,
            tc.tile_pool(name="phkx", bufs=2) as pkx,
            tc.tile_pool(name="phkps", bufs=1, space="PSUM") as pkps,
        ):
            wh, wl = load_w_hilo(pkw, pkst, wk_d)
            for jc in range(4):
                csl = slice(jc * 512, (jc + 1) * 512)
                xh, xl = load_xt_hilo(pkx, jc)
                ps = [pkps.tile([128, 512], F32, name=f"ps{a}", tag=f"ps{a}") for a in range(8)]
                pass_3term(wh, wl, xh, xl, ps)
                for ac in range(8):
                    nc.vector.tensor_copy(KTh[ac][:, csl], ps[ac][:])
                    nc.vector.tensor_sub(KTl[ac][:, csl], ps[ac][:], KTh[ac][:, csl])

        # ---- Phase 2: attention per local row-block ----------------------
        with (
            tc.tile_pool(name="attn", bufs=2) as pa,
            tc.tile_pool(name="attn1", bufs=2) as pa1,
            tc.tile_pool(name="psS", bufs=2, space="PSUM") as psS,
            tc.tile_pool(name="psT", bufs=2, space="PSUM") as psT,
            tc.tile_pool(name="psO", bufs=2, space="PSUM") as psO,
        ):
            for l in range(8):
                chunks = CHUNKS[l]
                nch = len(chunks)
                W = nch * 512
                lsl = slice(l * 128, (l + 1) * 128)
                S_sb = pa.tile([128, 2048], F32, tag="S")
                for k, ch in enumerate(chunks):
                    ps = psS.tile([128, 512], F32, tag="ps")
                    csl = slice(ch * 512, (ch + 1) * 512)
                    for ac in range(8):
                        nc.tensor.matmul(
                            ps[:], QTh[ac][:, lsl], KTh[ac][:, csl],
                            start=(ac == 0), stop=False,
                        )
                        nc.tensor.matmul(
                            ps[:], QTh[ac][:, lsl], KTl[ac][:, csl],
                            start=False, stop=False,
                        )
                        nc.tensor.matmul(
                            ps[:], QTl[ac][:, lsl], KTh[ac][:, csl],
                            start=False, stop=(ac == 7),
                        )
                    mk = pa1.tile([128, 512], BF16, tag="mk")
                    nc.gpsimd.dma_start(mk[:], mask_d[lsl, csl])
                    nc.vector.tensor_add(S_sb[:, k * 512 : (k + 1) * 512], ps[:], mk[:])

                mx = pa1.tile([128, 1], F32, tag="mx")
                nc.vector.reduce_max(mx[:], S_sb[:, 0:W], axis=mybir.AxisListType.X)
                negb = pa1.tile([128, 1], F32, tag="negb")
                nc.vector.tensor_scalar_mul(negb[:], mx[:], -1.0 / 32.0)
                P_sb = pa.tile([128, 2048], F32, tag="P")
                rs = pa1.tile([128, 1], F32, tag="rs")
                nc.scalar.activation(
                    P_sb[:, 0:W],
                    S_sb[:, 0:W],
                    mybir.ActivationFunctionType.Exp,
                    bias=negb[:],
                    scale=1.0 / 32.0,
                    accum_out=rs[:],
                )

                oacc = [psO.tile([128, 512], F32, name=f"oacc{h}", tag=f"oacc{h}") for h in range(2)]
                nq = nch * 4
                for q in range(nq):
                    vj = chunks[q // 4] * 4 + (q % 4)
                    pst = psT.tile([128, 128], F32, tag="pst")
                    nc.tensor.transpose(
                        pst[:], P_sb[:, q * 128 : (q + 1) * 128], ident[:]
                    )
                    pt = pa1.tile([128, 128], BF16, tag="pt")
                    nc.vector.tensor_copy(pt[:], pst[:])
                    for half in range(2):
                        nc.tensor.matmul(
                            oacc[half][:],
                            pt[:],
                            V[vj][:, half * 512 : (half + 1) * 512],
                            start=(q == 0),
                            stop=(q == nq - 1),
                        )

                rec = pa1.tile([128, 1], F32, tag="rec")
                nc.vector.reciprocal(rec[:], rs[:])
                for half in range(2):
                    o_sb = pa1.tile([128, 512], F32, tag="o")
                    nc.vector.tensor_scalar_mul(o_sb[:], oacc[half][:], rec[:])
                    nc.sync.dma_start(
                        out_d[lsl, half * 512 : (half + 1) * 512],
                        o_sb[:],
                    )

    nc.compile()
    _CACHE["nc"] = nc
    return nc


def _core_inputs(x, Wq, Wk, Wv, c):
    b = c // 2
    my = ABLK if c % 2 == 0 else BBLK
    perm = _perm_rows(my)
    gi = np.concatenate([np.arange(g * 128, (g + 1) * 128) for g in my])
    mask = np.where(perm[None, :] <= gi[:, None] + 1, 0.0, NEG).astype(
        ml_dtypes.bfloat16
    )
    return {
        "x_perm": np.ascontiguousarray(x[b][perm]),
        "wq": Wq,
        "wk": Wk,
        "wv": Wv,
        "maskb": mask,
    }, (b, my)


def kernel(x, Wq, Wk, Wv):
    x = np.ascontiguousarray(np.asarray(x, dtype=np.float32))
    Wq = np.ascontiguousarray(np.asarray(Wq, dtype=np.float32))
    Wk = np.ascontiguousarray(np.asarray(Wk, dtype=np.float32))
    Wv = np.ascontiguousarray(np.asarray(Wv, dtype=np.float32))

    nc = _build()

    in_maps = []
    metas = []
    for c in range(NCORES):
        m, meta = _core_inputs(x, Wq, Wk, Wv, c)
        in_maps.append(m)
        metas.append(meta)

    res = run_bass_kernel_spmd(nc, in_maps, list(range(NCORES)))

    out = np.empty((B, S, DA), dtype=np.float32)
    for c in range(NCORES):
        b, my = metas[c]
        o = res.results[c]["out"]
        for l, g in enumerate(my):
            out[b, g * 128 : (g + 1) * 128] = o[l * 128 : (l + 1) * 128]
    return out



# revision 6
# speedup vs baseline: 1.4172x; 1.4172x over previous
"""Causal attention (single head, d=1024) on 8 trn2 NeuronCores — v2.

out = softmax(mask(QK^T)/sqrt(1024)) @ V with mask j <= i+1,
x[4,2048,1024], Wq/Wk/Wv[1024,1024] fp32.

Sharding: 2 cores per batch; core handles 8 of 16 row-blocks
(A = {g%4 in {0,3}}, B = {g%4 in {1,2}} — consecutive (2p,2p+1) pairs
split one-each, balancing causal work). Host permutes x rows to
[own blocks | other blocks] and pre-transposes, so each core receives
xt = x[b][perm].T ([D,S], f32) plus Wq.T/Wk.T/Wv and small additive
mask windows; the program is identical on all cores (SPMD).

Algorithm (the big win vs v1): scores = x Wq (x Wk)^T = x G x^T with
G = Wq Wk^T [1024,1024]. Computing G (shared across all rows) replaces
the full-sequence K projection; T = x_own G then plays Q's role:
S = T x^T. Projections Q,K never materialize.

Precision: scores need ~2^-16 relative accuracy; G, T and S stages all
use 3-term split-bf16 matmuls (hi/lo decomposition). V and P (attention
weights) are single-term bf16 (~2^-9, ample under the 2e-2 gate).

Causality: per own row-block l the score strip is exactly
[own blocks 0..min(l+1,7)] + [other blocks 0..l] (union over the two
roles, 79 of 128 possible 128-col blocks instead of v1's 100), with
additive -1e30 mask windows (host data) on the <=3 non-trivial blocks.

P^T for the P@V matmul is produced by DMA xbar transposes (off the
tensor engine). No on-chip transposes remain at all.
"""

import numpy as np
import ml_dtypes

import concourse.bass as bass
import concourse.mybir as mybir
import concourse.tile as tile
from concourse import bacc
from concourse.bass_utils import run_bass_kernel_spmd
from contextlib import ExitStack

B, S, D, DA = 4, 2048, 1024, 1024
NCORES = 8
F32 = mybir.dt.float32
BF16 = mybir.dt.bfloat16

ABLK = [0, 3, 4, 7, 8, 11, 12, 15]
BBLK = [1, 2, 5, 6, 9, 10, 13, 14]
NEG = -1e30

OWN_W = [min(l + 2, 8) for l in range(8)]  # own-run width (128-blocks)
OTH_W = [l + 1 for l in range(8)]          # other-run width


def _strip_positions(l):
    """strip block index -> permuted 128-block position (= V tile index)."""
    return list(range(OWN_W[l])) + [8 + k for k in range(OTH_W[l])]


def _chunks(l):
    """S-matmul chunks: (xt_col_start, strip_col_start, width<=512)."""
    out = []
    ow = OWN_W[l] * 128
    for st in range(0, ow, 512):
        out.append((st, st, min(512, ow - st)))
    tw = OTH_W[l] * 128
    for st in range(0, tw, 512):
        out.append((1024 + st, ow + st, min(512, tw - st)))
    return out


_CACHE = {}


def _build():
    if "nc" in _CACHE:
        return _CACHE["nc"]

    nc = bacc.Bacc()
    xt_d = nc.dram_tensor("xt", [D, S], F32, kind="ExternalInput")
    wqt_d = nc.dram_tensor("wqt", [DA, D], F32, kind="ExternalInput")
    wkt_d = nc.dram_tensor("wkt", [DA, D], F32, kind="ExternalInput")
    wv_d = nc.dram_tensor("wv", [D, DA], F32, kind="ExternalInput")
    mka_d = nc.dram_tensor("mka", [1024, 256], BF16, kind="ExternalInput")
    mkb_d = nc.dram_tensor("mkb", [1024, 128], BF16, kind="ExternalInput")
    out_d = nc.dram_tensor("out", [1024, DA], F32, kind="ExternalOutput")

    with tile.TileContext(nc) as tc, ExitStack() as stack:
        # long-lived residents
        xpool = stack.enter_context(tc.tile_pool(name="xres", bufs=1))
        # xth[g][p, dp*512 + c] = bf16(x^T[dp*128+p, g*512+c]); xtl the residual
        xth = [xpool.tile([128, 4096], BF16, name=f"xth{g}", tag=f"xth{g}") for g in range(4)]
        xtl = [xpool.tile([128, 4096], BF16, name=f"xtl{g}", tag=f"xtl{g}") for g in range(4)]
        gpool = stack.enter_context(tc.tile_pool(name="gres", bufs=1))
        if True:
            # G[d1, d2] tiles [d1-part][128, 1024] (hi/lo)
            Gh = [gpool.tile([128, 1024], BF16, name=f"gh{d}", tag=f"gh{d}") for d in range(8)]
            Gl = [gpool.tile([128, 1024], BF16, name=f"gl{d}", tag=f"gl{d}") for d in range(8)]

            # ---- Phase G: G = Wq Wk^T via 3-term split-bf16 ----------------
            with (
                tc.tile_pool(name="wqk", bufs=1) as pw,
                tc.tile_pool(name="wf32", bufs=2) as pwf,
                tc.tile_pool(name="psG", bufs=3, space="PSUM") as psG,
            ):
                wqth = [pw.tile([128, D], BF16, name=f"wqth{a}", tag=f"wqth{a}") for a in range(8)]
                wqtl = [pw.tile([128, D], BF16, name=f"wqtl{a}", tag=f"wqtl{a}") for a in range(8)]
                wkth = [pw.tile([128, D], BF16, name=f"wkth{a}", tag=f"wkth{a}") for a in range(8)]
                wktl = [pw.tile([128, D], BF16, name=f"wktl{a}", tag=f"wktl{a}") for a in range(8)]
                for ap in range(8):
                    rsl = slice(ap * 128, (ap + 1) * 128)
                    nc.gpsimd.dma_start(wqth[ap][:], wqt_d[rsl, :])
                    wf = pwf.tile([128, D], F32, tag="wf")
                    nc.sync.dma_start(wf[:], wqt_d[rsl, :])
                    nc.vector.tensor_sub(wqtl[ap][:], wf[:], wqth[ap][:])
                    nc.gpsimd.dma_start(wkth[ap][:], wkt_d[rsl, :])
                    wf2 = pwf.tile([128, D], F32, tag="wf2")
                    nc.sync.dma_start(wf2[:], wkt_d[rsl, :])
                    nc.vector.tensor_sub(wktl[ap][:], wf2[:], wkth[ap][:])

                # x^T hi casts early on the gpsimd queue (behind W casts)
                for g in range(4):
                    for dp in range(8):
                        nc.gpsimd.dma_start(
                            xth[g][:, dp * 512 : (dp + 1) * 512],
                            xt_d[dp * 128 : (dp + 1) * 128, g * 512 : (g + 1) * 512],
                        )

                for d1 in range(8):
                    d1sl = slice(d1 * 128, (d1 + 1) * 128)
                    for half in range(2):
                        hsl = slice(half * 512, (half + 1) * 512)
                        ps = psG.tile([128, 512], F32, tag="psg")
                        for ap in range(8):
                            nc.tensor.matmul(ps[:], wqth[ap][:, d1sl], wkth[ap][:, hsl], start=(ap == 0), stop=False)
                            nc.tensor.matmul(ps[:], wqth[ap][:, d1sl], wktl[ap][:, hsl], start=False, stop=False)
                            nc.tensor.matmul(ps[:], wqtl[ap][:, d1sl], wkth[ap][:, hsl], start=False, stop=(ap == 7))
                        nc.vector.tensor_copy(Gh[d1][:, hsl], ps[:])
                        nc.vector.tensor_sub(Gl[d1][:, hsl], ps[:], Gh[d1][:, hsl])

            # T^T[d, i] for own rows i (hi/lo): tiles [d-part][128, 1024]
            # (allocated after the W pools close so addresses are reused)
            tpool = stack.enter_context(tc.tile_pool(name="tres", bufs=1))
            Th = [tpool.tile([128, 1024], BF16, name=f"th{d}", tag=f"th{d}") for d in range(8)]
            Tl = [tpool.tile([128, 1024], BF16, name=f"tl{d}", tag=f"tl{d}") for d in range(8)]

            # ---- x^T lo residuals (needed from phase T on) -----------------
            with tc.tile_pool(name="xf32", bufs=4) as pxf:
                for g in range(4):
                    for dp in range(8):
                        dsl = slice(dp * 512, (dp + 1) * 512)
                        xf = pxf.tile([128, 512], F32, tag="xf")
                        nc.scalar.dma_start(xf[:], xt_d[dp * 128 : (dp + 1) * 128, g * 512 : (g + 1) * 512])
                        nc.vector.tensor_sub(xtl[g][:, dsl], xf[:], xth[g][:, dsl])

                # ---- Phase T: T^T = (x_own G)^T, own rows = groups 0,1 -----
                with tc.tile_pool(name="psT", bufs=3, space="PSUM") as psT:
                    for dpp in range(8):
                        dsl = slice(dpp * 128, (dpp + 1) * 128)
                        for half in range(2):
                            hsl = slice(half * 512, (half + 1) * 512)
                            ps = psT.tile([128, 512], F32, tag="pst")
                            for dp in range(8):
                                rh = xth[half][:, dp * 512 : (dp + 1) * 512]
                                rl = xtl[half][:, dp * 512 : (dp + 1) * 512]
                                nc.tensor.matmul(ps[:], Gh[dp][:, dsl], rh, start=(dp == 0), stop=False)
                                nc.tensor.matmul(ps[:], Gh[dp][:, dsl], rl, start=False, stop=False)
                                nc.tensor.matmul(ps[:], Gl[dp][:, dsl], rh, start=False, stop=(dp == 7))
                            nc.vector.tensor_copy(Th[dpp][:, hsl], ps[:])
                            nc.vector.tensor_sub(Tl[dpp][:, hsl], ps[:], Th[dpp][:, hsl])

        # ---- Phase V: V = x @ Wv (single-term bf16) -----------------------
        vpool = stack.enter_context(tc.tile_pool(name="vres", bufs=1))
        V = [vpool.tile([128, DA], BF16, name=f"v{p}", tag=f"v{p}") for p in range(16)]
        with (
            tc.tile_pool(name="wvp", bufs=1) as pwv,
            tc.tile_pool(name="psV", bufs=4, space="PSUM") as psV,
        ):
            wvb = [pwv.tile([128, DA], BF16, name=f"wvb{d}", tag=f"wvb{d}") for d in range(8)]
            for dp in range(8):
                nc.gpsimd.dma_start(wvb[dp][:], wv_d[dp * 128 : (dp + 1) * 128, :])
            for g in range(4):
                for q in range(4):
                    p = g * 4 + q
                    for half in range(2):
                        ps = psV.tile([128, 512], F32, tag="psv")
                        for dp in range(8):
                            nc.tensor.matmul(
                                ps[:],
                                xth[g][:, dp * 512 + q * 128 : dp * 512 + (q + 1) * 128],
                                wvb[dp][:, half * 512 : (half + 1) * 512],
                                start=(dp == 0),
                                stop=(dp == 7),
                            )
                        nc.vector.tensor_copy(V[p][:, half * 512 : (half + 1) * 512], ps[:])

        # ---- Phase attn: per own row-block l (software-pipelined) ---------
        with (
            tc.tile_pool(name="pa", bufs=2) as pa,
            tc.tile_pool(name="pa1", bufs=3) as pa1,
            tc.tile_pool(name="psS", bufs=3, space="PSUM") as psS,
            tc.tile_pool(name="psO", bufs=2, space="PSUM") as psO,
        ):
            state = {}

            def emit_S(l):
                W = OWN_W[l] + OTH_W[l]
                S_sb = pa.tile([128, 2048], F32, tag="S")
                for (xc0, sc0, w) in _chunks(l):
                    g, off = divmod(xc0, 512)
                    ps = psS.tile([128, 512], F32, tag="ps")
                    for dp in range(8):
                        lh = Th[dp][:, l * 128 : (l + 1) * 128]
                        ll = Tl[dp][:, l * 128 : (l + 1) * 128]
                        rh = xth[g][:, dp * 512 + off : dp * 512 + off + w]
                        rl = xtl[g][:, dp * 512 + off : dp * 512 + off + w]
                        nc.tensor.matmul(ps[:, :w], lh, rh, start=(dp == 0), stop=False)
                        nc.tensor.matmul(ps[:, :w], lh, rl, start=False, stop=False)
                        nc.tensor.matmul(ps[:, :w], ll, rh, start=False, stop=(dp == 7))
                    nc.vector.tensor_copy(S_sb[:, sc0 : sc0 + w], ps[:, :w])
                mka = pa1.tile([128, 256], BF16, tag="mka")
                nc.gpsimd.dma_start(mka[:], mka_d[l * 128 : (l + 1) * 128, :])
                w1 = slice(l * 128, (l + 2) * 128)
                nc.vector.tensor_add(S_sb[:, w1], S_sb[:, w1], mka[:])
                mkb = pa1.tile([128, 128], BF16, tag="mkb")
                nc.gpsimd.dma_start(mkb[:], mkb_d[l * 128 : (l + 1) * 128, :])
                w2 = slice((W - 1) * 128, W * 128)
                nc.vector.tensor_add(S_sb[:, w2], S_sb[:, w2], mkb[:])

                mx = pa1.tile([128, 1], F32, tag="mx")
                nc.vector.reduce_max(mx[:], S_sb[:, : W * 128], axis=mybir.AxisListType.X)
                negb = pa1.tile([128, 1], F32, tag="negb")
                nc.vector.tensor_scalar_mul(negb[:], mx[:], -1.0 / 32.0)
                P_sb = pa.tile([128, 2048], BF16, tag="P")
                rs = pa1.tile([128, 1], F32, tag="rs")
                nc.scalar.activation(
                    P_sb[:, : W * 128],
                    S_sb[:, : W * 128],
                    mybir.ActivationFunctionType.Exp,
                    bias=negb[:],
                    scale=1.0 / 32.0,
                    accum_out=rs[:],
                )
                PT = pa.tile([128, 2048], BF16, tag="PT")
                eng = [nc.sync, nc.scalar]
                for b in range(W):
                    bsl = slice(b * 128, (b + 1) * 128)
                    eng[b % 2].dma_start_transpose(PT[:, bsl], P_sb[:, bsl])
                state[l] = (W, PT, rs)

            def emit_PV(l):
                W, PT, rs = state.pop(l)
                pos = _strip_positions(l)
                oacc = [psO.tile([128, 512], F32, name=f"oacc{h}", tag=f"oacc{h}") for h in range(2)]
                for b in range(W):
                    vj = pos[b]
                    bsl = slice(b * 128, (b + 1) * 128)
                    for half in range(2):
                        nc.tensor.matmul(
                            oacc[half][:],
                            PT[:, bsl],
                            V[vj][:, half * 512 : (half + 1) * 512],
                            start=(b == 0),
                            stop=(b == W - 1),
                        )
                rec = pa1.tile([128, 1], F32, tag="rec")
                nc.vector.reciprocal(rec[:], rs[:])
                for half in range(2):
                    o_sb = pa1.tile([128, 512], F32, tag=f"o{half}")
                    nc.vector.tensor_scalar_mul(o_sb[:], oacc[half][:], rec[:])
                    nc.sync.dma_start(
                        out_d[l * 128 : (l + 1) * 128, half * 512 : (half + 1) * 512],
                        o_sb[:],
                    )

            for l in range(8):
                emit_S(l)
                if l >= 1:
                    emit_PV(l - 1)
            emit_PV(7)

    nc.compile()
    _CACHE["nc"] = nc
    return nc


def _core_inputs(x, Wq, Wk, Wv, c):
    b = c // 2
    my = ABLK if c % 2 == 0 else BBLK
    oth = BBLK if c % 2 == 0 else ABLK
    permrows = np.concatenate([np.arange(g * 128, (g + 1) * 128) for g in my + oth])
    xt = np.ascontiguousarray(x[b][permrows].T)

    mka = np.zeros((1024, 256), dtype=ml_dtypes.bfloat16)
    mkb = np.zeros((1024, 128), dtype=ml_dtypes.bfloat16)
    for l in range(8):
        gi = my[l] * 128 + np.arange(128)
        strip = [my[k] for k in range(OWN_W[l])] + [oth[k] for k in range(OTH_W[l])]
        W = len(strip)
        for t, blk in enumerate((strip[l], strip[l + 1])):
            gj = blk * 128 + np.arange(128)
            mka[l * 128 : (l + 1) * 128, t * 128 : (t + 1) * 128] = np.where(
                gj[None, :] <= gi[:, None] + 1, 0.0, NEG
            )
        blk = strip[W - 1]
        gj = blk * 128 + np.arange(128)
        mkb[l * 128 : (l + 1) * 128, :] = np.where(gj[None, :] <= gi[:, None] + 1, 0.0, NEG)
        # all other strip blocks must be fully allowed
        for p2, blk2 in enumerate(strip):
            if p2 in (l, l + 1, W - 1):
                continue
            assert blk2 * 128 + 127 <= my[l] * 128 + 1, (l, p2, blk2)

    return {
        "xt": xt,
        "wqt": np.ascontiguousarray(Wq.T),
        "wkt": np.ascontiguousarray(Wk.T),
        "wv": np.ascontiguousarray(Wv),
        "mka": mka,
        "mkb": mkb,
    }, (b, my)


def kernel(x, Wq, Wk, Wv):
    x = np.ascontiguousarray(np.asarray(x, dtype=np.float32))
    Wq = np.ascontiguousarray(np.asarray(Wq, dtype=np.float32))
    Wk = np.ascontiguousarray(np.asarray(Wk, dtype=np.float32))
    Wv = np.ascontiguousarray(np.asarray(Wv, dtype=np.float32))

    nc = _build()

    in_maps = []
    metas = []
    for c in range(NCORES):
        m, meta = _core_inputs(x, Wq, Wk, Wv, c)
        in_maps.append(m)
        metas.append(meta)

    res = run_bass_kernel_spmd(nc, in_maps, list(range(NCORES)))

    out = np.empty((B, S, DA), dtype=np.float32)
    for c in range(NCORES):
        b, my = metas[c]
        o = res.results[c]["out"]
        for l, g in enumerate(my):
            out[b, g * 128 : (g + 1) * 128] = o[l * 128 : (l + 1) * 128]
    return out


# revision 9
# speedup vs baseline: 1.5095x; 1.0651x over previous
"""Causal attention (single head, d=1024) on 8 trn2 NeuronCores — v3.

out = softmax(mask(QK^T)/sqrt(1024)) @ V with mask j <= i+1,
x[4,2048,1024], Wq/Wk/Wv[1024,1024] fp32.

Sharding: 2 cores per batch; core handles 8 of 16 row-blocks
(A = {g%4 in {0,3}}, B = {g%4 in {1,2}} — consecutive (2p,2p+1) pairs
split one-each, balancing causal work). The host permutes x rows to
[own blocks | other blocks], transposes, and pre-splits every operand
into bf16 hi/lo pairs, so the kernel does no transposes and no hi/lo
splitting of inputs at all; the program is identical on all cores
(SPMD), with per-core content in the data.

Algorithm: scores = x Wq (x Wk)^T = x G x^T with G = Wq Wk^T
[1024,1024]. Computing G (shared across all rows) replaces the
full-sequence K projection; T = x_own G then plays Q's role:
S = T x^T. Projections Q,K never materialize.

Precision: scores need ~2^-16 relative accuracy; G, T and S stages all
use 3-term split-bf16 matmuls (hi/lo decomposition). V and P (attention
weights) are single-term bf16 (~2^-9, ample under the 2e-2 gate).

Causality: per own row-block l the score strip is exactly
[own blocks 0..min(l+1,7)] + [other blocks 0..l] (union over the two
roles; 79 of 128 possible 128-col blocks), with additive -1e30 mask
windows (host data) on the <=3 non-trivial blocks. Attention row-blocks
run in descending-width order so the serial softmax/PV tail is short.

P^T for the P@V matmul is produced by DMA xbar transposes (off the
tensor engine).
"""

import numpy as np
import ml_dtypes

import concourse.bass as bass
import concourse.mybir as mybir
import concourse.tile as tile
from concourse import bacc
from concourse.bass_utils import run_bass_kernel_spmd
from contextlib import ExitStack

B, S, D, DA = 4, 2048, 1024, 1024
NCORES = 8
F32 = mybir.dt.float32
BF16 = mybir.dt.bfloat16

ABLK = [0, 3, 4, 7, 8, 11, 12, 15]
BBLK = [1, 2, 5, 6, 9, 10, 13, 14]
NEG = -1e30

OWN_W = [min(l + 2, 8) for l in range(8)]  # own-run width (128-blocks)
OTH_W = [l + 1 for l in range(8)]          # other-run width


def _strip_positions(l):
    """strip block index -> permuted 128-block position (= V tile index)."""
    return list(range(OWN_W[l])) + [8 + k for k in range(OTH_W[l])]


def _chunks(l):
    """S-matmul chunks: (xt_col_start, strip_col_start, width<=512)."""
    out = []
    ow = OWN_W[l] * 128
    for st in range(0, ow, 512):
        out.append((st, st, min(512, ow - st)))
    tw = OTH_W[l] * 128
    for st in range(0, tw, 512):
        out.append((1024 + st, ow + st, min(512, tw - st)))
    return out


_CACHE = {}


def _build():
    if "nc" in _CACHE:
        return _CACHE["nc"]

    nc = bacc.Bacc()
    xth_d = nc.dram_tensor("xth", [D, S], BF16, kind="ExternalInput")
    xtl_d = nc.dram_tensor("xtl", [D, S], BF16, kind="ExternalInput")
    wqh_d = nc.dram_tensor("wqh", [DA, D], BF16, kind="ExternalInput")
    wql_d = nc.dram_tensor("wql", [DA, D], BF16, kind="ExternalInput")
    wkh_d = nc.dram_tensor("wkh", [DA, D], BF16, kind="ExternalInput")
    wkl_d = nc.dram_tensor("wkl", [DA, D], BF16, kind="ExternalInput")
    wvb_d = nc.dram_tensor("wvb", [D, DA], BF16, kind="ExternalInput")
    mka_d = nc.dram_tensor("mka", [1024, 256], BF16, kind="ExternalInput")
    mkb_d = nc.dram_tensor("mkb", [1024, 128], BF16, kind="ExternalInput")
    out_d = nc.dram_tensor("out", [1024, DA], F32, kind="ExternalOutput")

    with tile.TileContext(nc) as tc, ExitStack() as stack:
        # long-lived residents
        xpool = stack.enter_context(tc.tile_pool(name="xres", bufs=1))
        # xth[g][p, dp*512 + c] = bf16(x^T[dp*128+p, g*512+c]); xtl the residual
        xth = [xpool.tile([128, 4096], BF16, name=f"xth{g}", tag=f"xth{g}") for g in range(4)]
        xtl = [xpool.tile([128, 4096], BF16, name=f"xtl{g}", tag=f"xtl{g}") for g in range(4)]
        vpool = stack.enter_context(tc.tile_pool(name="vres", bufs=1))
        V = [vpool.tile([128, DA], BF16, name=f"v{p}", tag=f"v{p}") for p in range(16)]
        gpool = stack.enter_context(tc.tile_pool(name="gres", bufs=1))
        # G[d1, d2] tiles [d1-part][128, 1024] (hi/lo)
        Gh = [gpool.tile([128, 1024], BF16, name=f"gh{d}", tag=f"gh{d}") for d in range(8)]
        Gl = [gpool.tile([128, 1024], BF16, name=f"gl{d}", tag=f"gl{d}") for d in range(8)]
        # ---- input DMA (spread across queues; emission order = queue order)
        # gpsimd: wv, then x hi (V-phase order), then x lo
        # sync:   W hi/lo (needed ~60us in), later out stores
        # scalar: masks + PT transposes later
        with tc.tile_pool(name="wvp", bufs=1) as pwv:
            wvb = [pwv.tile([128, DA], BF16, name=f"wvb{d}", tag=f"wvb{d}") for d in range(8)]
            for dp in range(8):
                nc.gpsimd.dma_start(wvb[dp][:], wvb_d[dp * 128 : (dp + 1) * 128, :])
            for g in range(4):
                for dp in range(8):
                    nc.gpsimd.dma_start(
                        xth[g][:, dp * 512 : (dp + 1) * 512],
                        xth_d[dp * 128 : (dp + 1) * 128, g * 512 : (g + 1) * 512],
                    )
            for g in range(4):
                for dp in range(8):
                    nc.gpsimd.dma_start(
                        xtl[g][:, dp * 512 : (dp + 1) * 512],
                        xtl_d[dp * 128 : (dp + 1) * 128, g * 512 : (g + 1) * 512],
                    )
            # ---- Phase V: V = x @ Wv (single-term bf16) -------------------
            with tc.tile_pool(name="psV", bufs=4, space="PSUM") as psV:
                for g in range(4):
                    for q in range(4):
                        p = g * 4 + q
                        for half in range(2):
                            ps = psV.tile([128, 512], F32, tag="psv")
                            for dp in range(8):
                                nc.tensor.matmul(
                                    ps[:],
                                    xth[g][:, dp * 512 + q * 128 : dp * 512 + (q + 1) * 128],
                                    wvb[dp][:, half * 512 : (half + 1) * 512],
                                    start=(dp == 0),
                                    stop=(dp == 7),
                                )
                            nc.vector.tensor_copy(V[p][:, half * 512 : (half + 1) * 512], ps[:])

        # ---- Phase G: G = Wq Wk^T via 3-term split-bf16 -------------------
        # (W pool opens after wvp closes so SBUF fits; the W loads are the
        # first sync-queue work and so still execute from t~0)
        with tc.tile_pool(name="wqk", bufs=1) as pw, \
             tc.tile_pool(name="psG", bufs=3, space="PSUM") as psG:
            wqth = [pw.tile([128, D], BF16, name=f"wqth{a}", tag=f"wqth{a}") for a in range(8)]
            wqtl = [pw.tile([128, D], BF16, name=f"wqtl{a}", tag=f"wqtl{a}") for a in range(8)]
            wkth = [pw.tile([128, D], BF16, name=f"wkth{a}", tag=f"wkth{a}") for a in range(8)]
            wktl = [pw.tile([128, D], BF16, name=f"wktl{a}", tag=f"wktl{a}") for a in range(8)]
            for ap in range(8):
                rsl = slice(ap * 128, (ap + 1) * 128)
                nc.sync.dma_start(wqth[ap][:], wqh_d[rsl, :])
                nc.sync.dma_start(wqtl[ap][:], wql_d[rsl, :])
                nc.sync.dma_start(wkth[ap][:], wkh_d[rsl, :])
                nc.sync.dma_start(wktl[ap][:], wkl_d[rsl, :])
            for d1 in range(8):
                d1sl = slice(d1 * 128, (d1 + 1) * 128)
                for half in range(2):
                    hsl = slice(half * 512, (half + 1) * 512)
                    ps = psG.tile([128, 512], F32, tag="psg")
                    for ap in range(8):
                        nc.tensor.matmul(ps[:], wqth[ap][:, d1sl], wkth[ap][:, hsl], start=(ap == 0), stop=False)
                        nc.tensor.matmul(ps[:], wqth[ap][:, d1sl], wktl[ap][:, hsl], start=False, stop=False)
                        nc.tensor.matmul(ps[:], wqtl[ap][:, d1sl], wkth[ap][:, hsl], start=False, stop=(ap == 7))
                    nc.vector.tensor_copy(Gh[d1][:, hsl], ps[:])
                    nc.vector.tensor_sub(Gl[d1][:, hsl], ps[:], Gh[d1][:, hsl])

        # T^T[d, i] for own rows i (hi/lo): tiles [d-part][128, 1024]
        # (allocated after the W pool closes so addresses are reused)
        tpool = stack.enter_context(tc.tile_pool(name="tres", bufs=1))
        Th = [tpool.tile([128, 1024], BF16, name=f"th{d}", tag=f"th{d}") for d in range(8)]
        Tl = [tpool.tile([128, 1024], BF16, name=f"tl{d}", tag=f"tl{d}") for d in range(8)]

        # ---- Phase T: T^T = (x_own G)^T, own rows = groups 0,1 ------------
        with tc.tile_pool(name="psT", bufs=3, space="PSUM") as psT:
            for dpp in range(8):
                dsl = slice(dpp * 128, (dpp + 1) * 128)
                for half in range(2):
                    hsl = slice(half * 512, (half + 1) * 512)
                    ps = psT.tile([128, 512], F32, tag="pst")
                    for dp in range(8):
                        rh = xth[half][:, dp * 512 : (dp + 1) * 512]
                        rl = xtl[half][:, dp * 512 : (dp + 1) * 512]
                        nc.tensor.matmul(ps[:], Gh[dp][:, dsl], rh, start=(dp == 0), stop=False)
                        nc.tensor.matmul(ps[:], Gh[dp][:, dsl], rl, start=False, stop=False)
                        nc.tensor.matmul(ps[:], Gl[dp][:, dsl], rh, start=False, stop=(dp == 7))
                    nc.vector.tensor_copy(Th[dpp][:, hsl], ps[:])
                    nc.vector.tensor_sub(Tl[dpp][:, hsl], ps[:], Th[dpp][:, hsl])

        # ---- Phase attn: per own row-block l, descending strip width ------
        with (
            tc.tile_pool(name="pa", bufs=2) as pa,
            tc.tile_pool(name="pa1", bufs=3) as pa1,
            tc.tile_pool(name="psS", bufs=3, space="PSUM") as psS,
            tc.tile_pool(name="psO", bufs=2, space="PSUM") as psO,
        ):
            state = {}

            def emit_S(l):
                W = OWN_W[l] + OTH_W[l]
                S_sb = pa.tile([128, 2048], F32, name="S_sb", tag="S")
                for (xc0, sc0, w) in _chunks(l):
                    g, off = divmod(xc0, 512)
                    ps = psS.tile([128, 512], F32, name="ps", tag="ps")
                    for dp in range(8):
                        lh = Th[dp][:, l * 128 : (l + 1) * 128]
                        ll = Tl[dp][:, l * 128 : (l + 1) * 128]
                        rh = xth[g][:, dp * 512 + off : dp * 512 + off + w]
                        rl = xtl[g][:, dp * 512 + off : dp * 512 + off + w]
                        nc.tensor.matmul(ps[:, :w], lh, rh, start=(dp == 0), stop=False)
                        nc.tensor.matmul(ps[:, :w], lh, rl, start=False, stop=False)
                        nc.tensor.matmul(ps[:, :w], ll, rh, start=False, stop=(dp == 7))
                    nc.vector.tensor_copy(S_sb[:, sc0 : sc0 + w], ps[:, :w])
                mka = pa1.tile([128, 256], BF16, name="mka", tag="mka")
                nc.scalar.dma_start(mka[:], mka_d[l * 128 : (l + 1) * 128, :])
                w1 = slice(l * 128, (l + 2) * 128)
                nc.vector.tensor_add(S_sb[:, w1], S_sb[:, w1], mka[:])
                mkb = pa1.tile([128, 128], BF16, name="mkb", tag="mkb")
                nc.scalar.dma_start(mkb[:], mkb_d[l * 128 : (l + 1) * 128, :])
                w2 = slice((W - 1) * 128, W * 128)
                nc.vector.tensor_add(S_sb[:, w2], S_sb[:, w2], mkb[:])

                mx = pa1.tile([128, 1], F32, name="mx", tag="mx")
                nc.vector.reduce_max(mx[:], S_sb[:, : W * 128], axis=mybir.AxisListType.X)
                negb = pa1.tile([128, 1], F32, name="negb", tag="negb")
                nc.vector.tensor_scalar_mul(negb[:], mx[:], -1.0 / 32.0)
                P_sb = pa.tile([128, 2048], BF16, name="P_sb", tag="P")
                rs = pa1.tile([128, 1], F32, name="rs", tag="rs")
                nc.scalar.activation(
                    P_sb[:, : W * 128],
                    S_sb[:, : W * 128],
                    mybir.ActivationFunctionType.Exp,
                    bias=negb[:],
                    scale=1.0 / 32.0,
                    accum_out=rs[:],
                )
                PT = pa.tile([128, 2048], BF16, name="PT", tag="PT")
                eng = [nc.sync, nc.scalar]
                for b in range(W):
                    bsl = slice(b * 128, (b + 1) * 128)
                    eng[b % 2].dma_start_transpose(PT[:, bsl], P_sb[:, bsl])
                state[l] = (W, PT, rs)

            def emit_PV(l):
                W, PT, rs = state.pop(l)
                pos = _strip_positions(l)
                oacc = [psO.tile([128, 512], F32, name=f"oacc{h}", tag=f"oacc{h}") for h in range(2)]
                for b in range(W):
                    vj = pos[b]
                    bsl = slice(b * 128, (b + 1) * 128)
                    for half in range(2):
                        nc.tensor.matmul(
                            oacc[half][:],
                            PT[:, bsl],
                            V[vj][:, half * 512 : (half + 1) * 512],
                            start=(b == 0),
                            stop=(b == W - 1),
                        )
                rec = pa1.tile([128, 1], F32, name="rec", tag="rec")
                nc.vector.reciprocal(rec[:], rs[:])
                for half in range(2):
                    o_sb = pa1.tile([128, 512], F32, name=f"o{half}", tag=f"o{half}")
                    nc.vector.tensor_scalar_mul(o_sb[:], oacc[half][:], rec[:])
                    nc.sync.dma_start(
                        out_d[l * 128 : (l + 1) * 128, half * 512 : (half + 1) * 512],
                        o_sb[:],
                    )

            order = list(range(7, -1, -1))  # descending strip width
            for i, l in enumerate(order):
                emit_S(l)
                if i >= 1:
                    emit_PV(order[i - 1])
            emit_PV(order[-1])

    nc.compile()
    _CACHE["nc"] = nc
    return nc


def _split_bf16(a):
    hi = a.astype(ml_dtypes.bfloat16)
    lo = (a - hi.astype(np.float32)).astype(ml_dtypes.bfloat16)
    return np.ascontiguousarray(hi), np.ascontiguousarray(lo)


def _core_inputs(x, Wq, Wk, Wv, c):
    b = c // 2
    my = ABLK if c % 2 == 0 else BBLK
    oth = BBLK if c % 2 == 0 else ABLK
    permrows = np.concatenate([np.arange(g * 128, (g + 1) * 128) for g in my + oth])
    xt = np.ascontiguousarray(x[b][permrows].T)
    xth, xtl = _split_bf16(xt)
    wqh, wql = _split_bf16(np.ascontiguousarray(Wq.T))
    wkh, wkl = _split_bf16(np.ascontiguousarray(Wk.T))

    mka = np.zeros((1024, 256), dtype=ml_dtypes.bfloat16)
    mkb = np.zeros((1024, 128), dtype=ml_dtypes.bfloat16)
    for l in range(8):
        gi = my[l] * 128 + np.arange(128)
        strip = [my[k] for k in range(OWN_W[l])] + [oth[k] for k in range(OTH_W[l])]
        W = len(strip)
        for t, blk in enumerate((strip[l], strip[l + 1])):
            gj = blk * 128 + np.arange(128)
            mka[l * 128 : (l + 1) * 128, t * 128 : (t + 1) * 128] = np.where(
                gj[None, :] <= gi[:, None] + 1, 0.0, NEG
            )
        blk = strip[W - 1]
        gj = blk * 128 + np.arange(128)
        mkb[l * 128 : (l + 1) * 128, :] = np.where(gj[None, :] <= gi[:, None] + 1, 0.0, NEG)
        for p2, blk2 in enumerate(strip):
            if p2 in (l, l + 1, W - 1):
                continue
            assert blk2 * 128 + 127 <= my[l] * 128 + 1, (l, p2, blk2)

    return {
        "xth": xth,
        "xtl": xtl,
        "wqh": wqh,
        "wql": wql,
        "wkh": wkh,
        "wkl": wkl,
        "wvb": Wv.astype(ml_dtypes.bfloat16),
        "mka": mka,
        "mkb": mkb,
    }, (b, my)


def kernel(x, Wq, Wk, Wv):
    x = np.ascontiguousarray(np.asarray(x, dtype=np.float32))
    Wq = np.ascontiguousarray(np.asarray(Wq, dtype=np.float32))
    Wk = np.ascontiguousarray(np.asarray(Wk, dtype=np.float32))
    Wv = np.ascontiguousarray(np.asarray(Wv, dtype=np.float32))

    nc = _build()

    in_maps = []
    metas = []
    for c in range(NCORES):
        m, meta = _core_inputs(x, Wq, Wk, Wv, c)
        in_maps.append(m)
        metas.append(meta)

    res = run_bass_kernel_spmd(nc, in_maps, list(range(NCORES)))

    out = np.empty((B, S, DA), dtype=np.float32)
    for c in range(NCORES):
        b, my = metas[c]
        o = res.results[c]["out"]
        for l, g in enumerate(my):
            out[b, g * 128 : (g + 1) * 128] = o[l * 128 : (l + 1) * 128]
    return out
